# Initial kernel scaffold
#
"""Causal GQA attention block (QK L2-norm + RoPE) for 8 trn2 NeuronCores.

Sharding: tensor-parallel over head-halves (2) x data-parallel over batch (4).
Core c handles batch c//2 and heads [h*8, h*8+8) with h = c%2 (kv heads
[h*2, h*2+2)).  Each core computes its partial output-projection
out_part^T = w_o[:, cols].T-contraction; the host sums the two partials per
batch and transposes back.

Layouts on device (chosen so every reduction is a TensorE contraction over
partitions and every softmax op is a cheap per-partition ACT/DVE op):
  - x^T, Q^T, K^T: [feature(d), token(t)]  (d on partitions)
  - V:             [token, feature]        (t on partitions)
  - scores^T:      [k, q] so AV needs no transpose; softmax row-sums come
                   from an all-ones matmul; the max-subtraction is skipped
                   because QK-norm bounds scores to +-0.0884.
  - RoPE rotate-half is a signed 128x128 permutation matmul plus two
    elementwise multiplies with host-provided cos/sin tables.
All matmuls run in float32r (fast PE mode).
"""

import numpy as np

import concourse.mybir as mybir
import concourse.tile as tile
from concourse import bacc
from concourse import bass2jax

F32 = mybir.dt.float32
F32R = mybir.dt.float32r
AF = mybir.ActivationFunctionType

P = 128
B, T, D = 4, 2048, 2048
N_HEADS, HEAD_DIM, N_KV = 16, 128, 4
Q_DIM = N_HEADS * HEAD_DIM          # 2048
KV_DIM = N_KV * HEAD_DIM            # 512
H_Q = 8                             # q heads per core
H_KV = 2                            # kv heads per core
EQ = H_Q * HEAD_DIM                 # 1024 q features per core
EKV = H_KV * HEAD_DIM               # 256
SCALE = 0.08838834764831845
THETA = 10000.0

KSUB = D // P                       # 16 contraction subtiles
N_CORES = 8
TT_HALF = T // 2                    # 1024, phase-1 token half
NT512 = T // 512                    # 4 512-token tiles
NTB = T // P                        # 16 128-token blocks


def _build_module():
    nc = bacc.Bacc("TRN2", target_bir_lowering=False, debug=False)

    xt = nc.dram_tensor("xt", [D, T], F32R, kind="ExternalInput")
    wq = nc.dram_tensor("wq", [H_Q, P, KSUB, P], F32R, kind="ExternalInput")
    wk = nc.dram_tensor("wk", [P, KSUB, EKV], F32R, kind="ExternalInput")
    wv = nc.dram_tensor("wv", [P, KSUB, EKV], F32R, kind="ExternalInput")
    wo = nc.dram_tensor("wo", [P, H_Q, D], F32R, kind="ExternalInput")
    cos_t = nc.dram_tensor("cos_t", [P, T], F32R, kind="ExternalInput")
    sin_t = nc.dram_tensor("sin_t", [P, T], F32R, kind="ExternalInput")
    ones_m = nc.dram_tensor("ones_m", [P, P], F32R, kind="ExternalInput")
    pswap = nc.dram_tensor("pswap", [P, P], F32R, kind="ExternalInput")
    out_t = nc.dram_tensor("out_t", [D, T], F32, kind="ExternalOutput")

    with tile.TileContext(nc) as tc:
        with (
            tc.tile_pool(name="persist", bufs=1) as persist,
            tc.tile_pool(name="kv_persist", bufs=1) as kvp,
            tc.tile_pool(name="qdram", bufs=1, space="DRAM") as qdram,
            # attention-critical sbuf pools, pre-allocated so their
            # addresses never overlap phase-1 pools
            tc.tile_pool(name="qstream", bufs=3) as qstream,
            tc.tile_pool(name="att_sb", bufs=5) as att_sb,
        ):
            ones_sb = persist.tile([P, P], F32R)
            psw_sb = persist.tile([P, P], F32R)
            nc.sync.dma_start(ones_sb[:], ones_m.ap())
            nc.sync.dma_start(psw_sb[:], pswap.ap())
            k_sb = kvp.tile([P, H_KV, T], F32R)       # roped+normed K^T slabs
            v_sb = kvp.tile([P, NTB, EKV], F32R)      # V in [t, e] layout
            q_scr = [
                [
                    qdram.tile([P, 512], F32R, name=f"qscr_{h}_{t}")
                    for t in range(NT512)
                ]
                for h in range(H_Q)
            ]

            # ---------------- phase 1: qkv proj + L2 norm + rope ----------
            with (
                tc.tile_pool(name="xres", bufs=1) as xres,
                tc.tile_pool(name="wstream", bufs=2) as wstream,
                tc.tile_pool(name="wvres", bufs=1) as wvres,
                tc.tile_pool(name="p1tmp", bufs=2) as p1tmp,
                tc.tile_pool(name="p1out", bufs=2) as p1out,
                tc.tile_pool(name="trig", bufs=1) as trig,
                tc.tile_pool(name="pp", bufs=2, space="PSUM") as pp,
                tc.tile_pool(name="pssq", bufs=2, space="PSUM") as pssq,
                tc.tile_pool(name="psw", bufs=2, space="PSUM") as psw,
                tc.tile_pool(name="pv", bufs=2, space="PSUM") as pv,
            ):
                cos_sb = trig.tile([P, T], F32R)
                sin_sb = trig.tile([P, T], F32R)
                wv_sb = wvres.tile([P, KSUB, EKV], F32R)
                # K weights resident up front: the first projections are K,
                # and their lhsT must not queue behind the x-tile DMAs
                wk_sb = wvres.tile([P, KSUB, EKV], F32R, name="wk_sb")
                nc.sync.dma_start(wk_sb[:], wk.ap())
                for th in range(2):
                    t0 = th * TT_HALF
                    x_sb = xres.tile([P, KSUB, TT_HALF], F32R, tag="x")
                    xr = xt.ap()[:, t0 : t0 + TT_HALF].rearrange(
                        "(ks p) t -> p ks t", p=P
                    )
                    for ks in range(KSUB):
                        nc.sync.dma_start(x_sb[:, ks], xr[:, ks])
                    if th == 0:
                        # needed only from the first norm/rope (~35us in) and
                        # V projections; keep them behind the x stream
                        nc.sync.dma_start(cos_sb[:], cos_t.ap())
                        nc.sync.dma_start(sin_sb[:], sin_t.ap())
                        nc.sync.dma_start(wv_sb[:], wv.ap())

                    def proj_norm_rope(es):
                        """project feature block es, normalize, rope"""
                        if es < H_Q:
                            w_sb = wstream.tile([P, KSUB, P], F32R, tag="w")
                            nc.sync.dma_start(w_sb[:], wq.ap()[es])
                            w_use = w_sb[:]
                        else:
                            e0 = (es - H_Q) * P
                            w_use = wk_sb[:, :, e0 : e0 + P]
                        for tt in range(2):
                            tg = t0 + tt * 512
                            sl = slice(tt * 512, (tt + 1) * 512)
                            raw_ps = pp.tile([P, 512], F32, tag="raw")
                            for ks in range(KSUB):
                                nc.tensor.matmul(
                                    raw_ps[:],
                                    w_use[:, ks],
                                    x_sb[:, ks, sl],
                                    start=(ks == 0),
                                    stop=(ks == KSUB - 1),
                                )
                            sq = p1tmp.tile([P, 512], F32R, tag="t1")
                            nc.scalar.activation(sq[:], raw_ps[:], AF.Square)
                            ssq_ps = pssq.tile([P, 512], F32, tag="ssq")
                            nc.tensor.matmul(
                                ssq_ps[:], ones_sb[:], sq[:], start=True, stop=True
                            )
                            s_sb = p1tmp.tile([P, 512], F32, tag="t2")
                            nc.scalar.activation(s_sb[:], ssq_ps[:], AF.Sqrt)
                            r_sb = p1tmp.tile([P, 512], F32, tag="t3")
                            nc.vector.reciprocal_approx_fast(r_sb[:], s_sb[:])
                            qn = p1tmp.tile([P, 512], F32R, tag="t4")
                            nc.vector.tensor_mul(qn[:], raw_ps[:], r_sb[:])
                            ys = p1tmp.tile([P, 512], F32R, tag="t1")
                            nc.vector.tensor_mul(
                                ys[:], qn[:], sin_sb[:, tg : tg + 512]
                            )
                            sw_ps = psw.tile([P, 512], F32, tag="sw")
                            nc.tensor.matmul(
                                sw_ps[:], psw_sb[:], ys[:], start=True, stop=True
                            )
                            qc = p1tmp.tile([P, 512], F32, tag="t2")
                            nc.vector.tensor_mul(
                                qc[:], qn[:], cos_sb[:, tg : tg + 512]
                            )
                            if es < H_Q:
                                rope = p1out.tile([P, 512], F32R, tag="rope")
                                nc.vector.tensor_add(rope[:], sw_ps[:], qc[:])
                                nc.sync.dma_start(
                                    q_scr[es][tg // 512][:], rope[:]
                                )
                            else:
                                nc.vector.tensor_add(
                                    k_sb[:, es - H_Q, tg : tg + 512],
                                    sw_ps[:],
                                    qc[:],
                                )

                    # K first so attention can start earliest, then Q, then V
                    for es in (H_Q, H_Q + 1):
                        proj_norm_rope(es)
                    for es in range(H_Q):
                        proj_norm_rope(es)
                    for tb in range(TT_HALF // P):
                        tbg = th * (TT_HALF // P) + tb
                        v_ps = pv.tile([P, EKV], F32, tag="vp")
                        for ks in range(KSUB):
                            nc.tensor.matmul(
                                v_ps[:],
                                x_sb[:, ks, tb * P : (tb + 1) * P],
                                wv_sb[:, ks],
                                start=(ks == 0),
                                stop=(ks == KSUB - 1),
                            )
                        nc.scalar.copy(v_sb[:, tbg], v_ps[:])

            # ------- phase 2: attention + output projection per q-tile ----
            with (
                tc.tile_pool(name="wores", bufs=1) as wores,
                tc.tile_pool(name="p2tmp", bufs=2) as p2tmp,
                tc.tile_pool(name="oall", bufs=2) as oall,
                tc.tile_pool(name="fout", bufs=3) as fout,
                tc.tile_pool(name="psc", bufs=2, space="PSUM") as psc,
                tc.tile_pool(name="pav", bufs=1, space="PSUM") as pav,
                tc.tile_pool(name="psum2", bufs=1, space="PSUM") as psum2,
                tc.tile_pool(name="pf", bufs=2, space="PSUM") as pf,
            ):
                # w_o via the (idle) gpsimd DMA queue, split per slab, so it
                # never head-of-line-blocks the sync queue's q-tile loads
                wo_sb = wores.tile([P, H_Q, D], F32R)
                for ei in range(H_Q):
                    nc.gpsimd.dma_start(wo_sb[:, ei], wo.ap()[:, ei])
                for qt in range(NT512):
                    q0 = qt * 512
                    nkb = (qt + 1) * 4
                    o_all = oall.tile([P, H_Q, 512], F32R, tag="oa")
                    for hd in range(H_Q):
                        kvi = hd // 4
                        q_t = qstream.tile([P, 512], F32R, tag="q")
                        nc.sync.dma_start(q_t[:], q_scr[hd][qt][:])
                        atts = []

                        def diag_off(kb):
                            # left columns of a diagonal block that are fully
                            # masked; only skip when >=256 wide remains so
                            # fp32r keeps its fast mode
                            off = kb * P - q0
                            return off if off in (P, 2 * P) else 0

                        for kb0 in range(0, nkb, 2):
                            npair = min(2, nkb - kb0)
                            sc_ps = psc.tile([P, 1024], F32, tag="sc")
                            att = att_sb.tile([P, 1024], F32R, tag="att")
                            for j in range(npair):
                                kb = kb0 + j
                                off = diag_off(kb)
                                nc.tensor.matmul(
                                    sc_ps[:, j * 512 + off : (j + 1) * 512],
                                    k_sb[:, kvi, kb * P : (kb + 1) * P],
                                    q_t[:, off:],
                                    start=True,
                                    stop=True,
                                )
                            offs = [diag_off(kb0 + j) for j in range(npair)]
                            if not any(offs):
                                nc.scalar.activation(
                                    att[:, : npair * 512],
                                    sc_ps[:, : npair * 512],
                                    AF.Exp,
                                    scale=SCALE,
                                )
                            else:
                                for j in range(npair):
                                    sl = slice(j * 512 + offs[j], (j + 1) * 512)
                                    nc.scalar.activation(
                                        att[:, sl], sc_ps[:, sl], AF.Exp,
                                        scale=SCALE,
                                    )
                            for j in range(npair):
                                kb = kb0 + j
                                off = offs[j]
                                # zero future positions on diagonal blocks
                                if q0 < (kb + 1) * P and kb * P < q0 + 512:
                                    sl = slice(j * 512 + off, (j + 1) * 512)
                                    nc.gpsimd.affine_select(
                                        out=att[:, sl],
                                        in_=att[:, sl],
                                        compare_op=mybir.AluOpType.is_ge,
                                        fill=0.0,
                                        base=q0 + off - kb * P,
                                        pattern=[[1, 512 - off]],
                                        channel_multiplier=-1,
                                    )
                                atts.append((kb, att[:, j * 512 : (j + 1) * 512]))
                        o_ps = pav.tile([P, 512], F32, tag="av")
                        for kb, a_slice in atts:
                            off = diag_off(kb)
                            nc.tensor.matmul(
                                o_ps[:, off:],
                                v_sb[:, kb, kvi * HEAD_DIM : (kvi + 1) * HEAD_DIM],
                                a_slice[:, off:],
                                start=(kb == 0),
                                stop=(kb == nkb - 1),
                            )
                        s_ps = psum2.tile([P, 512], F32, tag="sum")
                        for kb, a_slice in atts:
                            off = diag_off(kb)
                            nc.tensor.matmul(
                                s_ps[:, off:],
                                ones_sb[:],
                                a_slice[:, off:],
                                start=(kb == 0),
                                stop=(kb == nkb - 1),
                            )
                        rs = p2tmp.tile([P, 512], F32, tag="rs")
                        nc.vector.reciprocal_approx_fast(rs[:], s_ps[:])
                        nc.vector.tensor_mul(o_all[:, hd], o_ps[:], rs[:])
                    for eo in range(D // P):
                        f_ps = pf.tile([P, 512], F32, tag="f")
                        for ei in range(H_Q):
                            nc.tensor.matmul(
                                f_ps[:],
                                wo_sb[:, ei, eo * P : (eo + 1) * P],
                                o_all[:, ei],
                                start=(ei == 0),
                                stop=(ei == H_Q - 1),
                            )
                        f_sb = fout.tile([P, 512], F32, tag="fo")
                        nc.scalar.copy(f_sb[:], f_ps[:])
                        nc.sync.dma_start(
                            out_t.ap()[eo * P : (eo + 1) * P, q0 : q0 + 512],
                            f_sb[:],
                        )

    nc.compile()
    return nc


def _re3(a):
    """[K, E] -> [P, K//P, E] host rearrange for contiguous weight DMAs."""
    return np.ascontiguousarray(a.reshape(-1, P, a.shape[1]).transpose(1, 0, 2))


def _host_inputs(x, w_qkv, w_o):
    """Build the 8 per-core input maps from full inputs."""
    x = np.asarray(x, dtype=np.float32)
    w_qkv = np.asarray(w_qkv, dtype=np.float32)
    w_o = np.asarray(w_o, dtype=np.float32)

    # rope tables, replicated on both 64-halves of the head dim
    half = HEAD_DIM // 2
    inv_freq = 1.0 / (
        THETA ** (np.arange(0, HEAD_DIM, 2, dtype=np.float32) / HEAD_DIM)
    )
    ang = np.arange(T, dtype=np.float32)[:, None] * inv_freq[None, :]  # [T, 64]
    cos = np.cos(ang).T.astype(np.float32)  # [64, T]
    sin = np.sin(ang).T.astype(np.float32)
    cos_t = np.ascontiguousarray(np.concatenate([cos, cos], axis=0))  # [128, T]
    sin_t = np.ascontiguousarray(np.concatenate([sin, sin], axis=0))

    ones_m = np.ones((P, P), dtype=np.float32)
    pswap = np.zeros((P, P), dtype=np.float32)
    for p in range(half):
        pswap[p, p + half] = 1.0    # out[m=p+64] += ys[p]
        pswap[p + half, p] = -1.0   # out[m=p]    -= ys[p+64]

    in_maps = []
    for c in range(N_CORES):
        b, h = c // 2, c % 2
        qrows = slice(h * EQ, (h + 1) * EQ)
        krows = slice(Q_DIM + h * EKV, Q_DIM + (h + 1) * EKV)
        vrows = slice(Q_DIM + KV_DIM + h * EKV, Q_DIM + (h + 1) * EKV + KV_DIM)
        wq_r = _re3(np.ascontiguousarray(w_qkv[qrows].T))     # [P, 16, 1024]
        wq_r4 = np.ascontiguousarray(
            wq_r.reshape(P, KSUB, H_Q, P).transpose(2, 0, 1, 3)
        )  # [H_Q, P, 16, 128]
        in_maps.append(
            {
                "xt": np.ascontiguousarray(x[b].T),
                "wq": wq_r4,
                "wk": _re3(np.ascontiguousarray(w_qkv[krows].T)),
                "wv": _re3(np.ascontiguousarray(w_qkv[vrows].T)),
                "wo": _re3(
                    np.ascontiguousarray(w_o[:, h * EQ : (h + 1) * EQ].T)
                ).reshape(P, H_Q, D),
                "cos_t": cos_t,
                "sin_t": sin_t,
                "ones_m": ones_m,
                "pswap": pswap,
            }
        )
    return in_maps


def _gather(results):
    out = np.empty((B, T, D), dtype=np.float32)
    for b in range(B):
        acc = results[2 * b]["out_t"] + results[2 * b + 1]["out_t"]
        out[b] = acc.T
    return out


_NC_CACHE = []


def _get_module():
    if not _NC_CACHE:
        _NC_CACHE.append(_build_module())
    return _NC_CACHE[0]


def kernel(x, w_qkv, w_o):
    nc = _get_module()
    in_maps = _host_inputs(x, w_qkv, w_o)
    results = bass2jax.run_bass_via_pjrt(nc, in_maps, n_cores=N_CORES)
    return _gather(results)



# revision 1
# speedup vs baseline: 1.1185x; 1.1185x over previous
"""Causal GQA attention block (QK L2-norm + RoPE) for 8 trn2 NeuronCores.

Sharding: tensor-parallel over head-halves (2) x data-parallel over batch (4).
Core c handles batch c//2 and heads [h*8, h*8+8) with h = c%2 (kv heads
[h*2, h*2+2)).  Each core computes its partial output-projection
out_part^T = w_o[:, cols].T-contraction; the host sums the two partials per
batch and transposes back.

Layouts on device (chosen so every reduction is a TensorE contraction over
partitions and every softmax op is a cheap per-partition ACT/DVE op):
  - x^T, Q^T, K^T: [feature(d), token(t)]  (d on partitions)
  - V:             [token, feature]        (t on partitions)
  - scores^T:      [k, q] so AV needs no transpose; softmax row-sums come
                   from an all-ones matmul; the max-subtraction is skipped
                   because QK-norm bounds scores to +-0.0884.
  - RoPE rotate-half is a signed 128x128 permutation matmul plus two
    elementwise multiplies with host-provided cos/sin tables.
All matmuls run in float32r (fast PE mode).
"""

import numpy as np

import concourse.mybir as mybir
import concourse.tile as tile
from concourse import bacc
from concourse import bass2jax

F32 = mybir.dt.float32
F32R = mybir.dt.float32r
AF = mybir.ActivationFunctionType

P = 128
B, T, D = 4, 2048, 2048
N_HEADS, HEAD_DIM, N_KV = 16, 128, 4
Q_DIM = N_HEADS * HEAD_DIM          # 2048
KV_DIM = N_KV * HEAD_DIM            # 512
H_Q = 8                             # q heads per core
H_KV = 2                            # kv heads per core
EQ = H_Q * HEAD_DIM                 # 1024 q features per core
EKV = H_KV * HEAD_DIM               # 256
SCALE = 0.08838834764831845
THETA = 10000.0

KSUB = D // P                       # 16 contraction subtiles
N_CORES = 8
TT_HALF = T // 2                    # 1024, phase-1 token half
NT512 = T // 512                    # 4 512-token tiles
NTB = T // P                        # 16 128-token blocks


def _build_module():
    nc = bacc.Bacc("TRN2", target_bir_lowering=False, debug=False)

    xt = nc.dram_tensor("xt", [D, T], F32R, kind="ExternalInput")
    wq = nc.dram_tensor("wq", [H_Q, P, KSUB, P], F32R, kind="ExternalInput")
    wk = nc.dram_tensor("wk", [P, KSUB, EKV], F32R, kind="ExternalInput")
    wv = nc.dram_tensor("wv", [P, KSUB, EKV], F32R, kind="ExternalInput")
    wo = nc.dram_tensor("wo", [P, H_Q, D], F32R, kind="ExternalInput")
    cos_t = nc.dram_tensor("cos_t", [P, T], F32R, kind="ExternalInput")
    sin_t = nc.dram_tensor("sin_t", [P, T], F32R, kind="ExternalInput")
    ones_m = nc.dram_tensor("ones_m", [P, P], F32R, kind="ExternalInput")
    pswap = nc.dram_tensor("pswap", [P, P], F32R, kind="ExternalInput")
    out_t = nc.dram_tensor("out_t", [D, T], F32, kind="ExternalOutput")

    with tile.TileContext(nc) as tc:
        with (
            tc.tile_pool(name="persist", bufs=1) as persist,
            tc.tile_pool(name="kv_persist", bufs=1) as kvp,
            tc.tile_pool(name="qdram", bufs=1, space="DRAM") as qdram,
            # attention-critical sbuf pools, pre-allocated so their
            # addresses never overlap phase-1 pools
            tc.tile_pool(name="qstream", bufs=3) as qstream,
            tc.tile_pool(name="att_sb", bufs=5) as att_sb,
        ):
            ones_sb = persist.tile([P, P], F32R)
            psw_sb = persist.tile([P, P], F32R)
            nc.sync.dma_start(ones_sb[:], ones_m.ap())
            nc.sync.dma_start(psw_sb[:], pswap.ap())
            k_sb = kvp.tile([P, H_KV, T], F32R)       # roped+normed K^T slabs
            v_sb = kvp.tile([P, NTB, EKV], F32R)      # V in [t, e] layout
            q_scr = [
                [
                    qdram.tile([P, 512], F32R, name=f"qscr_{h}_{t}")
                    for t in range(NT512)
                ]
                for h in range(H_Q)
            ]

            # ---------------- phase 1: qkv proj + L2 norm + rope ----------
            with (
                tc.tile_pool(name="xres", bufs=1) as xres,
                tc.tile_pool(name="wstream", bufs=2) as wstream,
                tc.tile_pool(name="wvres", bufs=1) as wvres,
                tc.tile_pool(name="p1tmp", bufs=2) as p1tmp,
                tc.tile_pool(name="p1out", bufs=2) as p1out,
                tc.tile_pool(name="trig", bufs=1) as trig,
                tc.tile_pool(name="pp", bufs=2, space="PSUM") as pp,
                tc.tile_pool(name="pssq", bufs=2, space="PSUM") as pssq,
                tc.tile_pool(name="psw", bufs=2, space="PSUM") as psw,
                tc.tile_pool(name="pv", bufs=2, space="PSUM") as pv,
            ):
                cos_sb = trig.tile([P, T], F32R)
                sin_sb = trig.tile([P, T], F32R)
                wv_sb = wvres.tile([P, KSUB, EKV], F32R)
                # K weights resident up front: the first projections are K,
                # and their lhsT must not queue behind the x-tile DMAs
                wk_sb = wvres.tile([P, KSUB, EKV], F32R, name="wk_sb")
                nc.sync.dma_start(wk_sb[:], wk.ap())
                for th in range(2):
                    t0 = th * TT_HALF
                    x_sb = xres.tile([P, KSUB, TT_HALF], F32R, tag="x")
                    xr = xt.ap()[:, t0 : t0 + TT_HALF].rearrange(
                        "(ks p) t -> p ks t", p=P
                    )
                    for ks in range(KSUB):
                        nc.sync.dma_start(x_sb[:, ks], xr[:, ks])
                    if th == 0:
                        # needed only from the first norm/rope (~35us in) and
                        # V projections; keep them behind the x stream
                        nc.sync.dma_start(cos_sb[:], cos_t.ap())
                        nc.sync.dma_start(sin_sb[:], sin_t.ap())
                        nc.sync.dma_start(wv_sb[:], wv.ap())

                    def proj_norm_rope(es):
                        """project feature block es, normalize, rope"""
                        if es < H_Q:
                            w_sb = wstream.tile([P, KSUB, P], F32R, tag="w")
                            nc.sync.dma_start(w_sb[:], wq.ap()[es])
                            w_use = w_sb[:]
                        else:
                            e0 = (es - H_Q) * P
                            w_use = wk_sb[:, :, e0 : e0 + P]
                        for tt in range(2):
                            tg = t0 + tt * 512
                            sl = slice(tt * 512, (tt + 1) * 512)
                            raw_ps = pp.tile([P, 512], F32, tag="raw")
                            for ks in range(KSUB):
                                nc.tensor.matmul(
                                    raw_ps[:],
                                    w_use[:, ks],
                                    x_sb[:, ks, sl],
                                    start=(ks == 0),
                                    stop=(ks == KSUB - 1),
                                )
                            sq = p1tmp.tile([P, 512], F32R, tag="t1")
                            nc.scalar.activation(sq[:], raw_ps[:], AF.Square)
                            ssq_ps = pssq.tile([P, 512], F32, tag="ssq")
                            nc.tensor.matmul(
                                ssq_ps[:], ones_sb[:], sq[:], start=True, stop=True
                            )
                            s_sb = p1tmp.tile([P, 512], F32, tag="t2")
                            nc.scalar.activation(s_sb[:], ssq_ps[:], AF.Sqrt)
                            r_sb = p1tmp.tile([P, 512], F32, tag="t3")
                            nc.vector.reciprocal_approx_fast(r_sb[:], s_sb[:])
                            qn = p1tmp.tile([P, 512], F32R, tag="t4")
                            nc.vector.tensor_mul(qn[:], raw_ps[:], r_sb[:])
                            ys = p1tmp.tile([P, 512], F32R, tag="t1")
                            nc.vector.tensor_mul(
                                ys[:], qn[:], sin_sb[:, tg : tg + 512]
                            )
                            sw_ps = psw.tile([P, 512], F32, tag="sw")
                            nc.tensor.matmul(
                                sw_ps[:], psw_sb[:], ys[:], start=True, stop=True
                            )
                            qc = p1tmp.tile([P, 512], F32, tag="t2")
                            nc.vector.tensor_mul(
                                qc[:], qn[:], cos_sb[:, tg : tg + 512]
                            )
                            if es < H_Q:
                                rope = p1out.tile([P, 512], F32R, tag="rope")
                                nc.vector.tensor_add(rope[:], sw_ps[:], qc[:])
                                nc.sync.dma_start(
                                    q_scr[es][tg // 512][:], rope[:]
                                )
                            else:
                                nc.vector.tensor_add(
                                    k_sb[:, es - H_Q, tg : tg + 512],
                                    sw_ps[:],
                                    qc[:],
                                )

                    # K first so attention can start earliest, then Q, then V
                    for es in (H_Q, H_Q + 1):
                        proj_norm_rope(es)
                    for es in range(H_Q):
                        proj_norm_rope(es)
                    for tb in range(TT_HALF // P):
                        tbg = th * (TT_HALF // P) + tb
                        v_ps = pv.tile([P, EKV], F32, tag="vp")
                        for ks in range(KSUB):
                            nc.tensor.matmul(
                                v_ps[:],
                                x_sb[:, ks, tb * P : (tb + 1) * P],
                                wv_sb[:, ks],
                                start=(ks == 0),
                                stop=(ks == KSUB - 1),
                            )
                        nc.scalar.copy(v_sb[:, tbg], v_ps[:])

            # ------- phase 2: attention + output projection per q-tile ----
            with (
                tc.tile_pool(name="wores", bufs=1) as wores,
                tc.tile_pool(name="p2tmp", bufs=2) as p2tmp,
                tc.tile_pool(name="oall", bufs=2) as oall,
                tc.tile_pool(name="fout", bufs=3) as fout,
                tc.tile_pool(name="psc", bufs=2, space="PSUM") as psc,
                tc.tile_pool(name="pav", bufs=1, space="PSUM") as pav,
                tc.tile_pool(name="psum2", bufs=1, space="PSUM") as psum2,
                tc.tile_pool(name="pf", bufs=2, space="PSUM") as pf,
            ):
                # w_o via the (idle) gpsimd DMA queue, split per slab, so it
                # never head-of-line-blocks the sync queue's q-tile loads
                wo_sb = wores.tile([P, H_Q, D], F32R)
                for ei in range(H_Q):
                    nc.gpsimd.dma_start(wo_sb[:, ei], wo.ap()[:, ei])
                for qt in range(NT512):
                    q0 = qt * 512
                    nkb = (qt + 1) * 4
                    o_all = oall.tile([P, H_Q, 512], F32R, tag="oa")
                    for hd in range(H_Q):
                        kvi = hd // 4
                        q_t = qstream.tile([P, 512], F32R, tag="q")
                        nc.sync.dma_start(q_t[:], q_scr[hd][qt][:])
                        atts = []

                        def diag_off(kb):
                            # left columns of a diagonal block that are fully
                            # masked; only skip when >=256 wide remains so
                            # fp32r keeps its fast mode
                            off = kb * P - q0
                            return off if off in (P, 2 * P) else 0

                        for kb0 in range(0, nkb, 2):
                            npair = min(2, nkb - kb0)
                            sc_ps = psc.tile([P, 1024], F32, tag="sc")
                            att = att_sb.tile([P, 1024], F32R, tag="att")
                            for j in range(npair):
                                kb = kb0 + j
                                off = diag_off(kb)
                                nc.tensor.matmul(
                                    sc_ps[:, j * 512 + off : (j + 1) * 512],
                                    k_sb[:, kvi, kb * P : (kb + 1) * P],
                                    q_t[:, off:],
                                    start=True,
                                    stop=True,
                                )
                            offs = [diag_off(kb0 + j) for j in range(npair)]
                            if not any(offs):
                                nc.scalar.activation(
                                    att[:, : npair * 512],
                                    sc_ps[:, : npair * 512],
                                    AF.Exp,
                                    scale=SCALE,
                                )
                            else:
                                for j in range(npair):
                                    sl = slice(j * 512 + offs[j], (j + 1) * 512)
                                    nc.scalar.activation(
                                        att[:, sl], sc_ps[:, sl], AF.Exp,
                                        scale=SCALE,
                                    )
                            for j in range(npair):
                                kb = kb0 + j
                                off = offs[j]
                                # zero future positions on diagonal blocks
                                if q0 < (kb + 1) * P and kb * P < q0 + 512:
                                    sl = slice(j * 512 + off, (j + 1) * 512)
                                    nc.gpsimd.affine_select(
                                        out=att[:, sl],
                                        in_=att[:, sl],
                                        compare_op=mybir.AluOpType.is_ge,
                                        fill=0.0,
                                        base=q0 + off - kb * P,
                                        pattern=[[1, 512 - off]],
                                        channel_multiplier=-1,
                                    )
                                atts.append((kb, att[:, j * 512 : (j + 1) * 512]))
                        o_ps = pav.tile([P, 512], F32, tag="av")
                        for kb, a_slice in atts:
                            off = diag_off(kb)
                            nc.tensor.matmul(
                                o_ps[:, off:],
                                v_sb[:, kb, kvi * HEAD_DIM : (kvi + 1) * HEAD_DIM],
                                a_slice[:, off:],
                                start=(kb == 0),
                                stop=(kb == nkb - 1),
                            )
                        s_ps = psum2.tile([P, 512], F32, tag="sum")
                        for kb, a_slice in atts:
                            off = diag_off(kb)
                            nc.tensor.matmul(
                                s_ps[:, off:],
                                ones_sb[:],
                                a_slice[:, off:],
                                start=(kb == 0),
                                stop=(kb == nkb - 1),
                            )
                        rs = p2tmp.tile([P, 512], F32, tag="rs")
                        nc.vector.reciprocal_approx_fast(rs[:], s_ps[:])
                        nc.vector.tensor_mul(o_all[:, hd], o_ps[:], rs[:])
                    for eo in range(D // P):
                        f_ps = pf.tile([P, 512], F32, tag="f")
                        for ei in range(H_Q):
                            nc.tensor.matmul(
                                f_ps[:],
                                wo_sb[:, ei, eo * P : (eo + 1) * P],
                                o_all[:, ei],
                                start=(ei == 0),
                                stop=(ei == H_Q - 1),
                            )
                        f_sb = fout.tile([P, 512], F32, tag="fo")
                        nc.scalar.copy(f_sb[:], f_ps[:])
                        nc.sync.dma_start(
                            out_t.ap()[eo * P : (eo + 1) * P, q0 : q0 + 512],
                            f_sb[:],
                        )

    nc.compile()
    return nc


def _re3(a):
    """[K, E] -> [P, K//P, E] host rearrange for contiguous weight DMAs."""
    return np.ascontiguousarray(a.reshape(-1, P, a.shape[1]).transpose(1, 0, 2))


def _host_inputs(x, w_qkv, w_o):
    """Build the 8 per-core input maps from full inputs."""
    x = np.asarray(x, dtype=np.float32)
    w_qkv = np.asarray(w_qkv, dtype=np.float32)
    w_o = np.asarray(w_o, dtype=np.float32)

    # rope tables, replicated on both 64-halves of the head dim
    half = HEAD_DIM // 2
    inv_freq = 1.0 / (
        THETA ** (np.arange(0, HEAD_DIM, 2, dtype=np.float32) / HEAD_DIM)
    )
    ang = np.arange(T, dtype=np.float32)[:, None] * inv_freq[None, :]  # [T, 64]
    cos = np.cos(ang).T.astype(np.float32)  # [64, T]
    sin = np.sin(ang).T.astype(np.float32)
    cos_t = np.ascontiguousarray(np.concatenate([cos, cos], axis=0))  # [128, T]
    sin_t = np.ascontiguousarray(np.concatenate([sin, sin], axis=0))

    ones_m = np.ones((P, P), dtype=np.float32)
    pswap = np.zeros((P, P), dtype=np.float32)
    for p in range(half):
        pswap[p, p + half] = 1.0    # out[m=p+64] += ys[p]
        pswap[p + half, p] = -1.0   # out[m=p]    -= ys[p+64]

    in_maps = []
    for c in range(N_CORES):
        b, h = c // 2, c % 2
        qrows = slice(h * EQ, (h + 1) * EQ)
        krows = slice(Q_DIM + h * EKV, Q_DIM + (h + 1) * EKV)
        vrows = slice(Q_DIM + KV_DIM + h * EKV, Q_DIM + (h + 1) * EKV + KV_DIM)
        wq_r = _re3(np.ascontiguousarray(w_qkv[qrows].T))     # [P, 16, 1024]
        wq_r4 = np.ascontiguousarray(
            wq_r.reshape(P, KSUB, H_Q, P).transpose(2, 0, 1, 3)
        )  # [H_Q, P, 16, 128]
        in_maps.append(
            {
                "xt": np.ascontiguousarray(x[b].T),
                "wq": wq_r4,
                "wk": _re3(np.ascontiguousarray(w_qkv[krows].T)),
                "wv": _re3(np.ascontiguousarray(w_qkv[vrows].T)),
                "wo": _re3(
                    np.ascontiguousarray(w_o[:, h * EQ : (h + 1) * EQ].T)
                ).reshape(P, H_Q, D),
                "cos_t": cos_t,
                "sin_t": sin_t,
                "ones_m": ones_m,
                "pswap": pswap,
            }
        )
    return in_maps


def _gather(results):
    out = np.empty((B, T, D), dtype=np.float32)
    for b in range(B):
        acc = results[2 * b]["out_t"] + results[2 * b + 1]["out_t"]
        out[b] = acc.T
    return out


_NC_CACHE = []


def _get_module():
    if not _NC_CACHE:
        _NC_CACHE.append(_build_module())
    return _NC_CACHE[0]


def kernel(x, w_qkv, w_o):
    nc = _get_module()
    in_maps = _host_inputs(x, w_qkv, w_o)
    results = bass2jax.run_bass_via_pjrt(nc, in_maps, n_cores=N_CORES)
    return _gather(results)

